# revision 7
# baseline (speedup 1.0000x reference)
"""Single-level 2D Haar DWT (periodization mode) on Trainium2.

Input x: (8, 512, 512, 16) fp32 NHWC. Output: (LL, LH, HL, HH), each
(8, 256, 256, 16) fp32 — +/- combinations of each 2x2 spatial block,
scaled by 0.5.

Sharding: pure data parallel — one batch sample per NeuronCore (8 cores).

The kernel is HBM-bandwidth bound (memory regime). All device I/O is
fp16: the host pre-scales x by 0.5 (exact) and downcasts to fp16
(rel err ~5e-4, tolerance is 2e-2), and upcasts the fp16 subband
outputs back to fp32. Per-core traffic is 16.8 MB (vs 33.6 MB fp32).
Input and output DMA streams ride separate directions/rings and
overlap, so the wall-clock target is set by the input stream plus
pipeline ramp, not in+out serialized.

Work is split by W-columns across two compute paths so no engine
paces below the DMA streams (x viewed per core as (512, 8192)):

Path A (cols 0:6144) — TensorE + ScalarE + VectorE:
  - TensorE: row-direction (H) butterfly as fp16 matmul with a fixed
    128x128 +/-1 weight (the 0.5 scale lives in the host prescale):
    PSUM rows 0..63 = top+bot, rows 64..127 = top-bot of each row pair.
  - ScalarE (ACT): PSUM -> SBUF copy with fp32 -> fp16 downcast.
  - VectorE: column (W) butterfly, fp16 tensor_tensor in 2x_1P mode:
    even +/- odd -> (LL|HL) and (LH|HH) tiles, 128 partitions each.

Path B (cols 6144:8192) — VectorE only: row pairs on partitions
(top/bot tiles), 2-op H butterfly then 4-op W butterfly, all fp16 2x.

Each subband gets its own DRAM output tensor (DMAs to one DRAM tensor
serialize against each other). Input DMAs ride the GpSimd SWDGE ring;
all output DMAs ride the Sync HWDGE ring — Sync has no compute duties,
so out-DMA semaphore waits cannot stall a compute FIFO.
"""

import sys

if "/opt/trn_rl_repo" not in sys.path:
    sys.path.insert(0, "/opt/trn_rl_repo")

import numpy as np

B, H, W, C = 8, 512, 512, 16
N_CORES = 8
HO, WO = H // 2, W // 2  # 256, 256
ROW = W * C  # 8192 elements per input row
OROW = WO * C  # 4096 elements per output row

A_W = 6144  # path A input columns (3 PSUM groups)
B_W = ROW - A_W  # 2048 path B input columns
A_OW = A_W // 2  # 3072 output columns from path A
GN = 2048  # PSUM group (4 banks)
MM_N = 512  # one matmul / PSUM bank

_CACHE = {}


def _haar_weight():
    """lhsT [k, m]: matmul computes out[m, n] = sum_k w[k, m] x[k, n]."""
    w = np.zeros((128, 128), dtype=np.float16)
    for m in range(64):
        w[2 * m, m] = 1.0
        w[2 * m + 1, m] = 1.0
        w[2 * m, 64 + m] = 1.0
        w[2 * m + 1, 64 + m] = -1.0
    return w


def _build():
    import concourse.bacc as bacc
    import concourse.mybir as mybir
    import concourse.tile as tile

    fp16 = mybir.dt.float16
    fp32 = mybir.dt.float32

    nc = bacc.Bacc(
        "TRN2", target_bir_lowering=False, debug=False, num_devices=N_CORES
    )
    x = nc.dram_tensor("x", (H, ROW), fp16, kind="ExternalInput")
    wdram = nc.dram_tensor("w", (128, 128), fp16, kind="ExternalInput")
    outs = {
        name: nc.dram_tensor(name, (HO, OROW), fp16, kind="ExternalOutput")
        for name in ("LL", "LH", "HL", "HH")
    }

    xq = x.rearrange("(q t) m -> q t m", t=2)  # [pair, row-parity, cols]

    def emit_a_unit(nc, pools, wt, kc):
        """Path A chunk kc: input rows kc*128..+128, cols 0:A_W."""
        inpA, psum, sbp, outA = pools
        rows = slice(kc * 128, (kc + 1) * 128)
        # two input tiles per chunk: the first PSUM group's matmuls only
        # wait on the leading 512KB, not the full 1.5MB
        xa = inpA.tile([128, GN], fp16, tag="xa")
        xb = inpA.tile([128, A_W - GN], fp16, tag="xb")
        nc.gpsimd.dma_start(xa[:], x[rows, 0:GN])
        nc.gpsimd.dma_start(xb[:], x[rows, GN:A_W])
        sb = sbp.tile([128, A_W], fp16)
        for g in range(A_W // GN):
            src = xa if g == 0 else xb
            soff = 0 if g == 0 else (g - 1) * GN
            ps = psum.tile([128, GN], fp32)
            for j in range(GN // MM_N):
                lo = j * MM_N
                nc.tensor.matmul(
                    ps[:, lo : lo + MM_N],
                    wt[:],
                    src[:, soff + lo : soff + lo + MM_N],
                    start=True,
                    stop=True,
                )
            nc.scalar.copy(sb[:, g * GN : (g + 1) * GN], ps[:])
        sum_t = outA.tile([128, A_OW], fp16, tag="sum")
        diff_t = outA.tile([128, A_OW], fp16, tag="diff")
        sv_in = sb[:].rearrange("p (w u c) -> p w u c", u=2, c=C)
        ev, od = sv_in[:, :, 0, :], sv_in[:, :, 1, :]
        sv = sum_t[:].rearrange("p (w c) -> p w c", c=C)
        dv = diff_t[:].rearrange("p (w c) -> p w c", c=C)
        nc.vector.tensor_add(sv, ev, od)
        nc.vector.tensor_sub(dv, ev, od)
        rs = slice(kc * 64, (kc + 1) * 64)
        cols = slice(0, A_OW)
        nc.sync.dma_start(outs["LL"][rs, cols], sum_t[0:64, :])
        nc.sync.dma_start(outs["HL"][rs, cols], sum_t[64:128, :])
        nc.sync.dma_start(outs["LH"][rs, cols], diff_t[0:64, :])
        nc.sync.dma_start(outs["HH"][rs, cols], diff_t[64:128, :])

    def emit_b_input(nc, inpB, pc):
        """Issue path B's input DMAs on the Scalar HWDGE ring (a second
        input ring running in parallel with GpSimd's SWDGE; these issues
        never wait, so they cannot stall ACT copies queued behind them)."""
        top = inpB.tile([128, B_W], fp16, tag="top")
        bot = inpB.tile([128, B_W], fp16, tag="bot")
        qs = slice(pc * 128, (pc + 1) * 128)
        ws = slice(A_W, ROW)
        nc.scalar.dma_start(top[:], xq[qs, 0, ws])
        nc.scalar.dma_start(bot[:], xq[qs, 1, ws])
        return top, bot

    def emit_b_unit(nc, pools, pc, top, bot):
        """Path B: 128 row-pairs pc*128..+128, input cols A_W:ROW."""
        midB, outB = pools
        qs = slice(pc * 128, (pc + 1) * 128)
        sum_b = midB.tile([128, B_W], fp16, tag="sum")
        diff_b = midB.tile([128, B_W], fp16, tag="diff")
        nc.vector.tensor_add(sum_b[:], top[:], bot[:])
        nc.vector.tensor_sub(diff_b[:], top[:], bot[:])
        WQ = B_W // (2 * C)  # 64 W-pairs
        otiles = {}
        for name, src, op in (
            ("LL", sum_b, "add"),
            ("LH", sum_b, "sub"),
            ("HL", diff_b, "add"),
            ("HH", diff_b, "sub"),
        ):
            s_in = src[:].rearrange("p (w u c) -> p w u c", u=2, c=C)
            ev, od = s_in[:, :, 0, :], s_in[:, :, 1, :]
            ot = outB.tile([128, WQ, C], fp16, tag=name)
            if op == "add":
                nc.vector.tensor_add(ot[:], ev, od)
            else:
                nc.vector.tensor_sub(ot[:], ev, od)
            otiles[name] = ot
        return otiles

    def emit_b_outs(nc, pc, otiles):
        """Path B output DMAs on the Scalar ring; call at a point where
        the B TTs are already done so the waits don't stall ACT copies."""
        qs = slice(pc * 128, (pc + 1) * 128)
        oc = slice(A_OW, A_OW + B_W // 2)
        for name, ot in otiles.items():
            nc.scalar.dma_start(
                outs[name][qs, oc],
                ot[:].rearrange("p w c -> p (w c)"),
            )

    with tile.TileContext(nc) as tc:
        with (
            tc.tile_pool(name="wpool", bufs=1) as wpool,
            tc.tile_pool(name="inpA", bufs=3) as inpA,
            tc.tile_pool(name="psum", bufs=2, space="PSUM") as psum,
            tc.tile_pool(name="sbp", bufs=2) as sbp,
            tc.tile_pool(name="outA", bufs=2) as outA,
            tc.tile_pool(name="inpB", bufs=2) as inpB,
            tc.tile_pool(name="midB", bufs=2) as midB,
            tc.tile_pool(name="outB", bufs=2) as outB,
        ):
            wt = wpool.tile([128, 128], fp16)
            nc.sync.dma_start(wt[:], wdram[:])
            a_pools = (inpA, psum, sbp, outA)
            b_pools = (midB, outB)
            # B inputs issue upfront on the Scalar ring (second input ring)
            b_in = [emit_b_input(nc, inpB, pc) for pc in range(2)]
            # interleave A and B units to keep DMA + all engines dense
            emit_a_unit(nc, a_pools, wt, 0)
            b0_outs = emit_b_unit(nc, b_pools, 0, *b_in[0])
            emit_a_unit(nc, a_pools, wt, 1)
            emit_b_outs(nc, 0, b0_outs)
            emit_a_unit(nc, a_pools, wt, 2)
            b1_outs = emit_b_unit(nc, b_pools, 1, *b_in[1])
            emit_a_unit(nc, a_pools, wt, 3)
            emit_b_outs(nc, 1, b1_outs)

    nc.compile()
    return nc


def _get_nc():
    if "nc" not in _CACHE:
        _CACHE["nc"] = _build()
    return _CACHE["nc"]


def _in_maps(x):
    w = _haar_weight()
    x16 = (x * np.float32(0.5)).astype(np.float16)
    return [
        {"x": np.ascontiguousarray(x16[i].reshape(H, ROW)), "w": w}
        for i in range(B)
    ]


def kernel(x):
    from concourse.bass_utils import run_bass_kernel_spmd

    x = np.asarray(x, dtype=np.float32)
    assert x.shape == (B, H, W, C), x.shape

    nc = _get_nc()
    try:
        res = run_bass_kernel_spmd(nc, _in_maps(x), list(range(N_CORES)))
    except Exception:
        # transient NRT device errors have been observed right after
        # compile; one retry has always succeeded
        res = run_bass_kernel_spmd(nc, _in_maps(x), list(range(N_CORES)))

    out = []
    for name in ("LL", "LH", "HL", "HH"):
        out.append(
            np.stack(
                [
                    res.results[i][name].astype(np.float32).reshape(HO, WO, C)
                    for i in range(B)
                ],
                axis=0,
            )
        )
    return tuple(out)


# revision 11
# speedup vs baseline: 1.0502x; 1.0502x over previous
"""Single-level 2D Haar DWT (periodization mode) on Trainium2.

Input x: (8, 512, 512, 16) fp32 NHWC. Output: (LL, LH, HL, HH), each
(8, 256, 256, 16) fp32 — +/- combinations of each 2x2 spatial block,
scaled by 0.5.

Sharding: pure data parallel — one batch sample per NeuronCore (8 cores).

The kernel is HBM-bandwidth bound (memory regime). All device I/O is
fp16: the host pre-scales x by 0.5 (exact) and downcasts to fp16
(rel err ~8e-4, tolerance is 2e-2), and upcasts the fp16 subband
outputs back to fp32. Per-core traffic is 16.8 MB; measured read and
write streams run concurrently (~330 GB/s each), so exec is set by
read-stream + pipeline tail, not in+out serialized.

Work splits by W-columns across two compute paths (x viewed per core
as (512, 8192)):

Path A (cols 0:6144) — TensorE + ScalarE + VectorE per 128-row chunk:
  matmul H-butterfly (fixed +/-1 128x128 fp16 weight) -> PSUM fp32 ->
  ACT copy-downcast to fp16 SBUF -> DVE W-butterfly (2x_1P mode).
  Chunk 0 and 3 load their input in two DMAs (2048+4096 cols) so the
  first PSUM group's matmuls start before the full 1.5MB lands.

Path B (cols 6144:8192) — VectorE only: row pairs on partitions,
2-op H butterfly then 4-op W butterfly, all fp16 2x.

Scheduling (the measured pacers are the write-stream start, the ACT
serial chain, and the Sync out-DMA issue chain):
  - B0's input DMAs go first: its outputs are ready ~5us after its
    data lands, so the output stream starts ~16us instead of ~25us.
  - B1 is mid-stream; A3 is last and split, so the final ACT chain and
    final DVE work end together instead of serializing.
  - A-path outputs issue right after the TT that produces them
    (LL/HL after the add, before the sub runs).
  - Inputs ride the GpSimd SWDGE ring; every output DMA rides the Sync
    HWDGE ring (Sync has no compute duties, so waits are harmless; a
    second HWDGE ring measurably slows both when used concurrently).
  - Each subband has its own DRAM tensor (same-tensor DMAs serialize).
"""

import sys

if "/opt/trn_rl_repo" not in sys.path:
    sys.path.insert(0, "/opt/trn_rl_repo")

import numpy as np

B, H, W, C = 8, 512, 512, 16
N_CORES = 8
HO, WO = H // 2, W // 2  # 256, 256
ROW = W * C  # 8192 elements per input row
OROW = WO * C  # 4096 elements per output row

A_W = 6144  # path A input columns (3 PSUM groups)
B_W = ROW - A_W  # 2048 path B input columns
A_OW = A_W // 2  # 3072 output columns from path A
GN = 2048  # PSUM group (4 banks)
MM_N = 512  # one matmul / PSUM bank

_CACHE = {}


def _haar_weight():
    """lhsT [k, m]: matmul computes out[m, n] = sum_k w[k, m] x[k, n]."""
    w = np.zeros((128, 128), dtype=np.float16)
    for m in range(64):
        w[2 * m, m] = 1.0
        w[2 * m + 1, m] = 1.0
        w[2 * m, 64 + m] = 1.0
        w[2 * m + 1, 64 + m] = -1.0
    return w


def _build():
    import concourse.bacc as bacc
    import concourse.mybir as mybir
    import concourse.tile as tile

    fp16 = mybir.dt.float16
    fp32 = mybir.dt.float32

    nc = bacc.Bacc(
        "TRN2", target_bir_lowering=False, debug=False, num_devices=N_CORES
    )
    x = nc.dram_tensor("x", (H, ROW), fp16, kind="ExternalInput")
    wdram = nc.dram_tensor("w", (128, 128), fp16, kind="ExternalInput")
    outs = {
        name: nc.dram_tensor(name, (HO, OROW), fp16, kind="ExternalOutput")
        for name in ("LL", "LH", "HL", "HH")
    }

    xq = x.rearrange("(q t) m -> q t m", t=2)  # [pair, row-parity, cols]

    def emit_a_unit(nc, pools, wt, kc):
        """Path A chunk kc: input rows kc*128..+128, cols 0:A_W.

        Input lands in two tiles (2048 + 4096 cols) so the first PSUM
        group's matmuls only wait on the leading 512KB of the chunk."""
        inpA, psum, sbp, outA = pools
        rows = slice(kc * 128, (kc + 1) * 128)
        xa = inpA.tile([128, GN], fp16, tag="xa")
        xb = inpA.tile([128, A_W - GN], fp16, tag="xb")
        nc.gpsimd.dma_start(xa[:], x[rows, 0:GN])
        nc.gpsimd.dma_start(xb[:], x[rows, GN:A_W])
        srcs = [(xa, 0), (xb, 0), (xb, GN)]
        sb = sbp.tile([128, A_W], fp16)
        for g, (src, soff) in enumerate(srcs):
            ps = psum.tile([128, GN], fp32)
            for j in range(GN // MM_N):
                lo = j * MM_N
                nc.tensor.matmul(
                    ps[:, lo : lo + MM_N],
                    wt[:],
                    src[:, soff + lo : soff + lo + MM_N],
                    start=True,
                    stop=True,
                )
            nc.scalar.copy(sb[:, g * GN : (g + 1) * GN], ps[:])
        sum_t = outA.tile([128, A_OW], fp16, tag="sum")
        diff_t = outA.tile([128, A_OW], fp16, tag="diff")
        sv_in = sb[:].rearrange("p (w u c) -> p w u c", u=2, c=C)
        ev, od = sv_in[:, :, 0, :], sv_in[:, :, 1, :]
        sv = sum_t[:].rearrange("p (w c) -> p w c", c=C)
        dv = diff_t[:].rearrange("p (w c) -> p w c", c=C)
        rs = slice(kc * 64, (kc + 1) * 64)
        cols = slice(0, A_OW)
        nc.vector.tensor_add(sv, ev, od)
        nc.sync.dma_start(outs["LL"][rs, cols], sum_t[0:64, :])
        nc.sync.dma_start(outs["HL"][rs, cols], sum_t[64:128, :])
        nc.vector.tensor_sub(dv, ev, od)
        nc.sync.dma_start(outs["LH"][rs, cols], diff_t[0:64, :])
        nc.sync.dma_start(outs["HH"][rs, cols], diff_t[64:128, :])

    def emit_b_input(nc, inpB, pc):
        top = inpB.tile([128, B_W], fp16, tag="top")
        bot = inpB.tile([128, B_W], fp16, tag="bot")
        qs = slice(pc * 128, (pc + 1) * 128)
        ws = slice(A_W, ROW)
        nc.gpsimd.dma_start(top[:], xq[qs, 0, ws])
        nc.gpsimd.dma_start(bot[:], xq[qs, 1, ws])
        return top, bot

    def emit_b_unit(nc, pools, pc, top, bot):
        """Path B: 128 row-pairs pc*128..+128, input cols A_W:ROW."""
        midB, outB = pools
        qs = slice(pc * 128, (pc + 1) * 128)
        sum_b = midB.tile([128, B_W], fp16, tag="sum")
        diff_b = midB.tile([128, B_W], fp16, tag="diff")
        nc.vector.tensor_add(sum_b[:], top[:], bot[:])
        nc.vector.tensor_sub(diff_b[:], top[:], bot[:])
        WQ = B_W // (2 * C)  # 64 W-pairs
        oc = slice(A_OW, A_OW + B_W // 2)
        for name, src, op in (
            ("LL", sum_b, "add"),
            ("LH", sum_b, "sub"),
            ("HL", diff_b, "add"),
            ("HH", diff_b, "sub"),
        ):
            s_in = src[:].rearrange("p (w u c) -> p w u c", u=2, c=C)
            ev, od = s_in[:, :, 0, :], s_in[:, :, 1, :]
            ot = outB.tile([128, WQ, C], fp16, tag=name)
            if op == "add":
                nc.vector.tensor_add(ot[:], ev, od)
            else:
                nc.vector.tensor_sub(ot[:], ev, od)
            nc.sync.dma_start(
                outs[name][qs, oc],
                ot[:].rearrange("p w c -> p (w c)"),
            )

    with tile.TileContext(nc) as tc:
        with (
            tc.tile_pool(name="wpool", bufs=1) as wpool,
            tc.tile_pool(name="inpA", bufs=3) as inpA,
            tc.tile_pool(name="psum", bufs=2, space="PSUM") as psum,
            tc.tile_pool(name="sbp", bufs=2) as sbp,
            tc.tile_pool(name="outA", bufs=2) as outA,
            tc.tile_pool(name="inpB", bufs=2) as inpB,
            tc.tile_pool(name="midB", bufs=2) as midB,
            tc.tile_pool(name="outB", bufs=2) as outB,
        ):
            wt = wpool.tile([128, 128], fp16)
            nc.sync.dma_start(wt[:], wdram[:])
            a_pools = (inpA, psum, sbp, outA)
            b_pools = (midB, outB)
            # input order = gpsimd emission order: B0 first (earliest
            # outputs), B1 mid-stream, A3 last (split, shortest tail)
            b0 = emit_b_input(nc, inpB, 0)
            emit_b_unit(nc, b_pools, 0, *b0)
            emit_a_unit(nc, a_pools, wt, 0)
            emit_a_unit(nc, a_pools, wt, 1)
            b1 = emit_b_input(nc, inpB, 1)
            emit_b_unit(nc, b_pools, 1, *b1)
            emit_a_unit(nc, a_pools, wt, 2)
            emit_a_unit(nc, a_pools, wt, 3)

    nc.compile()
    return nc


def _get_nc():
    if "nc" not in _CACHE:
        _CACHE["nc"] = _build()
    return _CACHE["nc"]


def _in_maps(x):
    w = _haar_weight()
    x16 = (x * np.float32(0.5)).astype(np.float16)
    return [
        {"x": np.ascontiguousarray(x16[i].reshape(H, ROW)), "w": w}
        for i in range(B)
    ]


def kernel(x):
    from concourse.bass_utils import run_bass_kernel_spmd

    x = np.asarray(x, dtype=np.float32)
    assert x.shape == (B, H, W, C), x.shape

    nc = _get_nc()
    try:
        res = run_bass_kernel_spmd(nc, _in_maps(x), list(range(N_CORES)))
    except Exception:
        # transient NRT device errors have been observed right after
        # compile; one retry has always succeeded
        res = run_bass_kernel_spmd(nc, _in_maps(x), list(range(N_CORES)))

    out = []
    for name in ("LL", "LH", "HL", "HH"):
        out.append(
            np.stack(
                [
                    res.results[i][name].astype(np.float32).reshape(HO, WO, C)
                    for i in range(B)
                ],
                axis=0,
            )
        )
    return tuple(out)


# revision 15
# speedup vs baseline: 1.1355x; 1.0812x over previous
"""Single-level 2D Haar DWT (periodization mode) on Trainium2.

Input x: (8, 512, 512, 16) fp32 NHWC. Output: (LL, LH, HL, HH), each
(8, 256, 256, 16) fp32 — +/- combinations of each 2x2 spatial block,
scaled by 0.5.

Sharding: pure data parallel — one batch sample per NeuronCore (8 cores).

The kernel is HBM-bandwidth bound (memory regime). All device I/O is
fp16: the host pre-scales x by 0.5 (exact) and downcasts to fp16
(rel err ~8e-4, tolerance is 2e-2), and upcasts the fp16 subband
outputs back to fp32. Per-core traffic is 16.8 MB; measured read and
write streams run concurrently (~330 GB/s each), so exec is set by
read-stream + pipeline tail, not in+out serialized.

Work splits by W-columns across two compute paths (x viewed per core
as (512, 8192)):

Path A (cols 0:6144) — TensorE + ScalarE + VectorE per 128-row chunk:
  matmul H-butterfly (fixed +/-1 128x128 fp16 weight) -> PSUM fp32 ->
  ACT copy-downcast to fp16 SBUF -> DVE W-butterfly (2x_1P mode).
  Chunk 0 and 3 load their input in two DMAs (2048+4096 cols) so the
  first PSUM group's matmuls start before the full 1.5MB lands.

Path B (cols 6144:8192) — VectorE only: row pairs on partitions,
2-op H butterfly then 4-op W butterfly, all fp16 2x.

Scheduling (the measured pacers are the write-stream start, the ACT
serial chain, and the Sync out-DMA issue chain):
  - B0's input DMAs go first: its outputs are ready ~5us after its
    data lands, so the output stream starts ~16us instead of ~25us.
  - B1 is mid-stream; A3 is last and split, so the final ACT chain and
    final DVE work end together instead of serializing.
  - A-path outputs issue right after the TT that produces them
    (LL/HL after the add, before the sub runs).
  - Inputs ride the GpSimd SWDGE ring; every output DMA rides the Sync
    HWDGE ring (Sync has no compute duties, so waits are harmless; a
    second HWDGE ring measurably slows both when used concurrently).
  - Each subband has its own DRAM tensor (same-tensor DMAs serialize).
"""

import sys

if "/opt/trn_rl_repo" not in sys.path:
    sys.path.insert(0, "/opt/trn_rl_repo")

import numpy as np

B, H, W, C = 8, 512, 512, 16
N_CORES = 8
HO, WO = H // 2, W // 2  # 256, 256
ROW = W * C  # 8192 elements per input row
OROW = WO * C  # 4096 elements per output row

A_W = 6144  # path A input columns (3 PSUM groups)
B_W = ROW - A_W  # 2048 path B input columns
A_OW = A_W // 2  # 3072 output columns from path A
GN = 2048  # PSUM group (4 banks)
MM_N = 512  # one matmul / PSUM bank

_CACHE = {}


def _haar_weight():
    """lhsT [k, m]: matmul computes out[m, n] = sum_k w[k, m] x[k, n]."""
    w = np.zeros((128, 128), dtype=np.float16)
    for m in range(64):
        w[2 * m, m] = 1.0
        w[2 * m + 1, m] = 1.0
        w[2 * m, 64 + m] = 1.0
        w[2 * m + 1, 64 + m] = -1.0
    return w


def _build():
    import concourse.bacc as bacc
    import concourse.mybir as mybir
    import concourse.tile as tile

    fp16 = mybir.dt.float16
    fp32 = mybir.dt.float32

    nc = bacc.Bacc(
        "TRN2", target_bir_lowering=False, debug=False, num_devices=N_CORES
    )
    x = nc.dram_tensor("x", (H, ROW), fp16, kind="ExternalInput")
    wdram = nc.dram_tensor("w", (128, 128), fp16, kind="ExternalInput")
    outs = {
        name: nc.dram_tensor(name, (HO, OROW), fp16, kind="ExternalOutput")
        for name in ("LL", "LH", "HL", "HH")
    }

    xq = x.rearrange("(q t) m -> q t m", t=2)  # [pair, row-parity, cols]

    def emit_a_unit(nc, pools, wt, kc, split_input=False):
        """Path A chunk kc: input rows kc*128..+128, cols 0:A_W.

        With split_input the chunk lands in two tiles (2048 + 4096
        cols) so the first PSUM group's matmuls only wait on the
        leading 512KB — used for the first chunk (starts the ACT chain
        sooner) and the last (shortens the post-stream tail)."""
        inpA, psum, sbp, outA = pools
        rows = slice(kc * 128, (kc + 1) * 128)
        xa = inpA.tile([128, GN], fp16, tag="xa")
        xb = inpA.tile([128, A_W - GN], fp16, tag="xb")
        nc.gpsimd.dma_start(xa[:], x[rows, 0:GN])
        nc.gpsimd.dma_start(xb[:], x[rows, GN:A_W])
        srcs = [(xa, 0), (xb, 0), (xb, GN)]
        sb = sbp.tile([128, A_W], fp16)
        for g, (src, soff) in enumerate(srcs):
            ps = psum.tile([128, GN], fp32)
            for j in range(GN // MM_N):
                lo = j * MM_N
                nc.tensor.matmul(
                    ps[:, lo : lo + MM_N],
                    wt[:],
                    src[:, soff + lo : soff + lo + MM_N],
                    start=True,
                    stop=True,
                )
            nc.scalar.copy(sb[:, g * GN : (g + 1) * GN], ps[:])
        sum_t = outA.tile([128, A_OW], fp16, tag="sum")
        diff_t = outA.tile([128, A_OW], fp16, tag="diff")
        sv_in = sb[:].rearrange("p (w u c) -> p w u c", u=2, c=C)
        ev, od = sv_in[:, :, 0, :], sv_in[:, :, 1, :]
        sv = sum_t[:].rearrange("p (w c) -> p w c", c=C)
        dv = diff_t[:].rearrange("p (w c) -> p w c", c=C)
        rs = slice(kc * 64, (kc + 1) * 64)
        cols = slice(0, A_OW)
        nc.vector.tensor_add(sv, ev, od)
        nc.sync.dma_start(outs["LL"][rs, cols], sum_t[0:64, :])
        nc.sync.dma_start(outs["HL"][rs, cols], sum_t[64:128, :])
        nc.vector.tensor_sub(dv, ev, od)
        nc.sync.dma_start(outs["LH"][rs, cols], diff_t[0:64, :])
        nc.sync.dma_start(outs["HH"][rs, cols], diff_t[64:128, :])

    def emit_b_input(nc, inpB, pc):
        top = inpB.tile([128, B_W], fp16, tag="top")
        bot = inpB.tile([128, B_W], fp16, tag="bot")
        qs = slice(pc * 128, (pc + 1) * 128)
        ws = slice(A_W, ROW)
        nc.gpsimd.dma_start(top[:], xq[qs, 0, ws])
        nc.gpsimd.dma_start(bot[:], xq[qs, 1, ws])
        return top, bot

    def emit_b_unit(nc, pools, pc, top, bot):
        """Path B: 128 row-pairs pc*128..+128, input cols A_W:ROW."""
        midB, outB = pools
        qs = slice(pc * 128, (pc + 1) * 128)
        sum_b = midB.tile([128, B_W], fp16, tag="sum")
        diff_b = midB.tile([128, B_W], fp16, tag="diff")
        nc.vector.tensor_add(sum_b[:], top[:], bot[:])
        nc.vector.tensor_sub(diff_b[:], top[:], bot[:])
        WQ = B_W // (2 * C)  # 64 W-pairs
        oc = slice(A_OW, A_OW + B_W // 2)
        for name, src, op in (
            ("LL", sum_b, "add"),
            ("LH", sum_b, "sub"),
            ("HL", diff_b, "add"),
            ("HH", diff_b, "sub"),
        ):
            s_in = src[:].rearrange("p (w u c) -> p w u c", u=2, c=C)
            ev, od = s_in[:, :, 0, :], s_in[:, :, 1, :]
            ot = outB.tile([128, WQ, C], fp16, tag=name)
            if op == "add":
                nc.vector.tensor_add(ot[:], ev, od)
            else:
                nc.vector.tensor_sub(ot[:], ev, od)
            nc.sync.dma_start(
                outs[name][qs, oc],
                ot[:].rearrange("p w c -> p (w c)"),
            )

    with tile.TileContext(nc) as tc:
        with (
            tc.tile_pool(name="wpool", bufs=1) as wpool,
            tc.tile_pool(name="inpA", bufs=4) as inpA,
            tc.tile_pool(name="psum", bufs=2, space="PSUM") as psum,
            tc.tile_pool(name="sbp", bufs=2) as sbp,
            tc.tile_pool(name="outA", bufs=2) as outA,
            tc.tile_pool(name="inpB", bufs=2) as inpB,
            tc.tile_pool(name="midB", bufs=2) as midB,
            tc.tile_pool(name="outB", bufs=2) as outB,
        ):
            wt = wpool.tile([128, 128], fp16)
            nc.sync.dma_start(wt[:], wdram[:])
            a_pools = (inpA, psum, sbp, outA)
            b_pools = (midB, outB)
            # unit order as in the best-measured schedule: A0 first so
            # the ACT chain starts as early as possible, B interleaved
            emit_a_unit(nc, a_pools, wt, 0)
            b0 = emit_b_input(nc, inpB, 0)
            emit_b_unit(nc, b_pools, 0, *b0)
            emit_a_unit(nc, a_pools, wt, 1)
            emit_a_unit(nc, a_pools, wt, 2)
            b1 = emit_b_input(nc, inpB, 1)
            emit_b_unit(nc, b_pools, 1, *b1)
            emit_a_unit(nc, a_pools, wt, 3)

    nc.compile()
    return nc


def _get_nc():
    if "nc" not in _CACHE:
        _CACHE["nc"] = _build()
    return _CACHE["nc"]


def _in_maps(x):
    w = _haar_weight()
    x16 = (x * np.float32(0.5)).astype(np.float16)
    return [
        {"x": np.ascontiguousarray(x16[i].reshape(H, ROW)), "w": w}
        for i in range(B)
    ]


def kernel(x):
    from concourse.bass_utils import run_bass_kernel_spmd

    x = np.asarray(x, dtype=np.float32)
    assert x.shape == (B, H, W, C), x.shape

    nc = _get_nc()
    try:
        res = run_bass_kernel_spmd(nc, _in_maps(x), list(range(N_CORES)))
    except Exception:
        # transient NRT device errors have been observed right after
        # compile; one retry has always succeeded
        res = run_bass_kernel_spmd(nc, _in_maps(x), list(range(N_CORES)))

    out = []
    for name in ("LL", "LH", "HL", "HH"):
        out.append(
            np.stack(
                [
                    res.results[i][name].astype(np.float32).reshape(HO, WO, C)
                    for i in range(B)
                ],
                axis=0,
            )
        )
    return tuple(out)


# revision 18
# speedup vs baseline: 1.1611x; 1.0225x over previous
"""Single-level 2D Haar DWT (periodization mode) on Trainium2.

Input x: (8, 512, 512, 16) fp32 NHWC. Output: (LL, LH, HL, HH), each
(8, 256, 256, 16) fp32 — +/- combinations of each 2x2 spatial block,
scaled by 0.5.

Sharding: pure data parallel — one batch sample per NeuronCore (8 cores).

The kernel is HBM-bandwidth bound (memory regime). All device I/O is
fp16: the host pre-scales x by 0.5 (exact) and downcasts to fp16
(rel err ~8e-4, tolerance is 2e-2), and upcasts the fp16 subband
outputs back to fp32. Per-core traffic is 16.8 MB; measured read and
write streams run concurrently (~330 GB/s each), so exec is set by
read-stream + pipeline tail, not in+out serialized.

Work splits by W-columns across two compute paths (x viewed per core
as (512, 8192)):

Path A (cols 0:6144) — TensorE + ScalarE + VectorE per 128-row chunk:
  matmul H-butterfly (fixed +/-1 128x128 fp16 weight) -> PSUM fp32 ->
  ACT copy-downcast to fp16 SBUF -> DVE W-butterfly (2x_1P mode).
  Chunk 0 and 3 load their input in two DMAs (2048+4096 cols) so the
  first PSUM group's matmuls start before the full 1.5MB lands.

Path B (cols 6144:8192) — VectorE only: row pairs on partitions,
2-op H butterfly then 4-op W butterfly, all fp16 2x.

Scheduling (the measured pacers are the write-stream start, the ACT
serial chain, and the Sync out-DMA issue chain):
  - B0's input DMAs go first: its outputs are ready ~5us after its
    data lands, so the output stream starts ~16us instead of ~25us.
  - B1 is mid-stream; A3 is last and split, so the final ACT chain and
    final DVE work end together instead of serializing.
  - A-path outputs issue right after the TT that produces them
    (LL/HL after the add, before the sub runs).
  - Inputs ride the GpSimd SWDGE ring; every output DMA rides the Sync
    HWDGE ring (Sync has no compute duties, so waits are harmless; a
    second HWDGE ring measurably slows both when used concurrently).
  - Each subband has its own DRAM tensor (same-tensor DMAs serialize).
"""

import sys

if "/opt/trn_rl_repo" not in sys.path:
    sys.path.insert(0, "/opt/trn_rl_repo")

import numpy as np

B, H, W, C = 8, 512, 512, 16
N_CORES = 8
HO, WO = H // 2, W // 2  # 256, 256
ROW = W * C  # 8192 elements per input row
OROW = WO * C  # 4096 elements per output row

A_W = 5120  # path A input columns (PSUM groups 2048+2048+1024)
B_W = ROW - A_W  # 3072 path B input columns
A_OW = A_W // 2  # 2560 output columns from path A
GN = 2048  # PSUM group (4 banks)
MM_N = 512  # one matmul / PSUM bank
A_GROUPS = (2048, 2048, 1024)

_CACHE = {}


def _haar_weight():
    """lhsT [k, m]: matmul computes out[m, n] = sum_k w[k, m] x[k, n]."""
    w = np.zeros((128, 128), dtype=np.float16)
    for m in range(64):
        w[2 * m, m] = 1.0
        w[2 * m + 1, m] = 1.0
        w[2 * m, 64 + m] = 1.0
        w[2 * m + 1, 64 + m] = -1.0
    return w


def _build():
    import concourse.bacc as bacc
    import concourse.mybir as mybir
    import concourse.tile as tile

    fp16 = mybir.dt.float16
    fp32 = mybir.dt.float32

    nc = bacc.Bacc(
        "TRN2", target_bir_lowering=False, debug=False, num_devices=N_CORES
    )
    x = nc.dram_tensor("x", (H, ROW), fp16, kind="ExternalInput")
    wdram = nc.dram_tensor("w", (128, 128), fp16, kind="ExternalInput")
    outs = {
        name: nc.dram_tensor(name, (HO, OROW), fp16, kind="ExternalOutput")
        for name in ("LL", "LH", "HL", "HH")
    }

    xq = x.rearrange("(q t) m -> q t m", t=2)  # [pair, row-parity, cols]

    def emit_a_unit(nc, pools, wt, kc, split_input=False):
        """Path A chunk kc: input rows kc*128..+128, cols 0:A_W.

        With split_input the chunk lands in two tiles (2048 + 4096
        cols) so the first PSUM group's matmuls only wait on the
        leading 512KB — used for the first chunk (starts the ACT chain
        sooner) and the last (shortens the post-stream tail)."""
        inpA, psum, sbp, outA = pools
        rows = slice(kc * 128, (kc + 1) * 128)
        xa = inpA.tile([128, GN], fp16, tag="xa")
        xb = inpA.tile([128, A_W - GN], fp16, tag="xb")
        nc.gpsimd.dma_start(xa[:], x[rows, 0:GN])
        nc.gpsimd.dma_start(xb[:], x[rows, GN:A_W])
        srcs = [(xa, 0), (xb, 0), (xb, GN)]
        sb = sbp.tile([128, A_W], fp16)
        goff = 0
        for (src, soff), gsz in zip(srcs, A_GROUPS):
            ps = psum.tile([128, GN], fp32)
            for j in range(gsz // MM_N):
                lo = j * MM_N
                nc.tensor.matmul(
                    ps[:, lo : lo + MM_N],
                    wt[:],
                    src[:, soff + lo : soff + lo + MM_N],
                    start=True,
                    stop=True,
                )
            nc.scalar.copy(sb[:, goff : goff + gsz], ps[:, 0:gsz])
            goff += gsz
        sum_t = outA.tile([128, A_OW], fp16, tag="sum")
        diff_t = outA.tile([128, A_OW], fp16, tag="diff")
        sv_in = sb[:].rearrange("p (w u c) -> p w u c", u=2, c=C)
        ev, od = sv_in[:, :, 0, :], sv_in[:, :, 1, :]
        sv = sum_t[:].rearrange("p (w c) -> p w c", c=C)
        dv = diff_t[:].rearrange("p (w c) -> p w c", c=C)
        rs = slice(kc * 64, (kc + 1) * 64)
        cols = slice(0, A_OW)
        nc.vector.tensor_add(sv, ev, od)
        nc.sync.dma_start(outs["LL"][rs, cols], sum_t[0:64, :])
        nc.sync.dma_start(outs["HL"][rs, cols], sum_t[64:128, :])
        nc.vector.tensor_sub(dv, ev, od)
        nc.sync.dma_start(outs["LH"][rs, cols], diff_t[0:64, :])
        nc.sync.dma_start(outs["HH"][rs, cols], diff_t[64:128, :])

    def emit_b_input(nc, inpB, pc):
        top = inpB.tile([128, B_W], fp16, tag="top")
        bot = inpB.tile([128, B_W], fp16, tag="bot")
        qs = slice(pc * 128, (pc + 1) * 128)
        ws = slice(A_W, ROW)
        nc.gpsimd.dma_start(top[:], xq[qs, 0, ws])
        nc.gpsimd.dma_start(bot[:], xq[qs, 1, ws])
        return top, bot

    def emit_b_unit(nc, pools, pc, top, bot):
        """Path B: 128 row-pairs pc*128..+128, input cols A_W:ROW."""
        midB, outB = pools
        qs = slice(pc * 128, (pc + 1) * 128)
        sum_b = midB.tile([128, B_W], fp16, tag="sum")
        diff_b = midB.tile([128, B_W], fp16, tag="diff")
        nc.vector.tensor_add(sum_b[:], top[:], bot[:])
        nc.vector.tensor_sub(diff_b[:], top[:], bot[:])
        WQ = B_W // (2 * C)  # 64 W-pairs
        oc = slice(A_OW, A_OW + B_W // 2)
        for name, src, op in (
            ("LL", sum_b, "add"),
            ("LH", sum_b, "sub"),
            ("HL", diff_b, "add"),
            ("HH", diff_b, "sub"),
        ):
            s_in = src[:].rearrange("p (w u c) -> p w u c", u=2, c=C)
            ev, od = s_in[:, :, 0, :], s_in[:, :, 1, :]
            ot = outB.tile([128, WQ, C], fp16, tag=name)
            if op == "add":
                nc.vector.tensor_add(ot[:], ev, od)
            else:
                nc.vector.tensor_sub(ot[:], ev, od)
            nc.sync.dma_start(
                outs[name][qs, oc],
                ot[:].rearrange("p w c -> p (w c)"),
            )

    with tile.TileContext(nc) as tc:
        with (
            tc.tile_pool(name="wpool", bufs=1) as wpool,
            tc.tile_pool(name="inpA", bufs=4) as inpA,
            tc.tile_pool(name="psum", bufs=2, space="PSUM") as psum,
            tc.tile_pool(name="sbp", bufs=2) as sbp,
            tc.tile_pool(name="outA", bufs=2) as outA,
            tc.tile_pool(name="inpB", bufs=2) as inpB,
            tc.tile_pool(name="midB", bufs=2) as midB,
            tc.tile_pool(name="outB", bufs=2) as outB,
        ):
            wt = wpool.tile([128, 128], fp16)
            nc.sync.dma_start(wt[:], wdram[:])
            a_pools = (inpA, psum, sbp, outA)
            b_pools = (midB, outB)
            # B0 first in both the read stream and the DVE queue: its
            # outputs are ready ~5us after its data lands, so the write
            # stream starts early; B1 second-to-last so the DVE is clear
            # for the final A chunk's butterflies
            b0 = emit_b_input(nc, inpB, 0)
            emit_b_unit(nc, b_pools, 0, *b0)
            emit_a_unit(nc, a_pools, wt, 0)
            emit_a_unit(nc, a_pools, wt, 1)
            emit_a_unit(nc, a_pools, wt, 2)
            b1 = emit_b_input(nc, inpB, 1)
            emit_b_unit(nc, b_pools, 1, *b1)
            emit_a_unit(nc, a_pools, wt, 3)

    nc.compile()
    return nc


def _get_nc():
    if "nc" not in _CACHE:
        _CACHE["nc"] = _build()
    return _CACHE["nc"]


def _in_maps(x):
    w = _haar_weight()
    x16 = (x * np.float32(0.5)).astype(np.float16)
    return [
        {"x": np.ascontiguousarray(x16[i].reshape(H, ROW)), "w": w}
        for i in range(B)
    ]


def kernel(x):
    from concourse.bass_utils import run_bass_kernel_spmd

    x = np.asarray(x, dtype=np.float32)
    assert x.shape == (B, H, W, C), x.shape

    nc = _get_nc()
    try:
        res = run_bass_kernel_spmd(nc, _in_maps(x), list(range(N_CORES)))
    except Exception:
        # transient NRT device errors have been observed right after
        # compile; one retry has always succeeded
        res = run_bass_kernel_spmd(nc, _in_maps(x), list(range(N_CORES)))

    out = []
    for name in ("LL", "LH", "HL", "HH"):
        out.append(
            np.stack(
                [
                    res.results[i][name].astype(np.float32).reshape(HO, WO, C)
                    for i in range(B)
                ],
                axis=0,
            )
        )
    return tuple(out)
